# revision 29
# baseline (speedup 1.0000x reference)
"""Trainium2 Bass kernel for BaselineProtonet (retrieval_knn).

logits[q, c] = -||query_q - proto_c||_2
  proto_c = mean of 64 support embeddings of class c
  embeddings_stacked: [64 classes * (64 support + 64 query), 1024] f32

Sharding (8 cores): query-sharded, support-replicated. Core i owns query
rows 512i..512(i+1); every core receives the full support set (fp8 on
the wire) and computes all 64 prototypes locally on the TensorEngine, so
no cross-core collective is needed (a ncfw collective costs ~50us of
control latency in this runtime, far more than the extra DMA).

Host-side shard prep (layout/encoding only, no arithmetic): support is
pre-swizzled to the exact SBUF layout (contiguous per-partition runs so
HWDGE descriptor generation is cheap) and encoded fp8e4m3; queries are
transposed to feature-major (d on partitions) and encoded bf16.

Per core:
  protos   : 64 one-hot matmuls (fp8) accumulate class sums -> PSUM
             [64,1024] f32, scaled 1/64 on evacuation -> bf16 prototypes
  P^T      : 8 PE transposes -> W = -2*P^T (bf16)
  ||p||^2  : DVE square + reduce on prototypes -> [64,1] f32, added
             per-partition (class) via the ACT sqrt bias
  ||q||^2  : DVE squares + ones-stationary colsum matmuls -> [1,512]
             f32, added via a K=1 fp32 matmul broadcast over classes
  Gram     : 8 accumulating matmuls lhsT=W chunk, rhs=Q^T chunk (bf16)
  logits   : -sqrt(dist^2) via ACT sqrt(+bias) and DVE negate,
             output [64, 512] (class-major); host transposes/concats.
PE is pre-warmed with dummy matmuls during the DMA wait (HAM clock gate)
and the sqrt ACT table is preloaded by a dummy activation.
"""

import numpy as np

C = 64          # classes
S = 64          # support per class (== queries per class)
D = 1024        # embedding dim
NCORES = 8
CL = C // NCORES            # 8 classes per core's query shard
QL = CL * S                 # 512 query rows per core
DCH = D // 128              # 8 d-chunks
SCH = (C * S) // 128        # 32 support row chunks (full support)

_CACHE = {}


def _emit(nc, tc, sup, qt, oh_in, out):
    """Emit the per-core tile program.

    sup:   [128, SCH*D] fp8 DRAM  (full support, swizzled: row p holds
                                   sup[j*128+p, :] for j = 0..31)
    qt:    [128, DCH*QL] bf16 DRAM (queries, swizzled feature-major)
    oh_in: [128, SCH*C] fp8 DRAM  (one-hot class masks per row chunk)
    out:   [C, QL] f32 DRAM       (negated distances, class-major)
    """
    from concourse import masks, mybir

    f32 = mybir.dt.float32
    bf16 = mybir.dt.bfloat16
    fp8 = mybir.dt.float8e4
    AF = mybir.ActivationFunctionType

    with (
        tc.tile_pool(name="sb", bufs=1) as sb,
        tc.tile_pool(name="ps", bufs=1, space="PSUM") as ps,
    ):
        # warm the PE clock first-thing (HAM gate needs ~3.5us of busy
        # before the real matmuls; deps are a single DVE memset)
        wm_in = sb.tile([128, 512], bf16)
        nc.vector.memset(wm_in[:], 0.0)
        wm_ps = ps.tile([128, 512], f32)
        for _ in range(8):
            nc.tensor.matmul(
                wm_ps[:], wm_in[:, 0:128], wm_in[:], start=True, stop=True
            )

        # ---------------- input DMAs (one sync-ring FIFO: one-hot, then
        # the support stream, then queries -- so the support slices drain
        # strictly in order and the prototype matmuls track the stream)
        oh = sb.tile([128, SCH, C], fp8)
        nc.sync.dma_start(oh[:], oh_in[:, :].rearrange("p (c k) -> p c k", c=SCH))
        s8 = sb.tile([128, SCH, D], fp8)
        for b in range(16):
            nc.sync.dma_start(
                s8[:, 2 * b : 2 * (b + 1)],
                sup[:, 2 * b * D : 2 * (b + 1) * D].rearrange(
                    "p (c d) -> p c d", c=2
                ),
            )
        q16 = sb.tile([128, DCH, QL], bf16)
        for h in range(2):
            nc.sync.dma_start(
                q16[:, 4 * h : 4 * (h + 1)],
                qt[:, 4 * h * QL : 4 * (h + 1) * QL].rearrange(
                    "p (k q) -> p k q", k=4
                ),
            )

        # ---------------- constants -------------------------------------
        ident = sb.tile([128, 128], bf16)
        masks.make_identity(nc, ident[:])
        ones64 = sb.tile([128, C], bf16)
        nc.gpsimd.memset(ones64[:], 1.0)

        # preload the sqrt ACT table set off the critical path
        warm_sq = sb.tile([1, 1], f32)
        nc.gpsimd.memset(warm_sq[:], 1.0)
        nc.scalar.activation(warm_sq[:], warm_sq[:], AF.Sqrt)

        # ---------------- prototypes (all 64 classes) -------------------
        # col-paired fp8 matmuls: the d-halves of each row chunk run
        # concurrently on disjoint column halves of the PE array
        # (rows 0..63 of p_ps2 = class sums for d 0..511, rows 64..127 =
        # class sums for d 512..1023)
        p_ps2 = ps.tile([128, 512], f32)
        for j in range(SCH):
            for h in range(2):
                nc.tensor.matmul(
                    p_ps2[64 * h : 64 * (h + 1), :],
                    oh[:, j],
                    s8[:, j, 512 * h : 512 * (h + 1)],
                    start=(j == 0),
                    stop=(j == SCH - 1),
                    tile_position=(0, 64 * h),
                    # the sim's psum group tracker is not partition-aware;
                    # h=0 carries the group accounting for the bank
                    skip_group_check=(h == 1),
                )
        psb = sb.tile([C, D], bf16)
        for h in range(2):
            nc.vector.tensor_scalar_mul(
                psb[:, 512 * h : 512 * (h + 1)], p_ps2[64 * h : 64 * (h + 1), :],
                1.0 / S,
            )

        # ---------------- W = -2 * P^T (bf16, ACT evac) ------------------
        pt_ps = ps.tile([128, DCH * C], bf16)  # chunk k at cols 64k..64k+64
        for k in range(DCH):
            nc.tensor.transpose(
                pt_ps[:, C * k : C * (k + 1)],
                psb[:, 128 * k : 128 * (k + 1)],
                ident[0:C, 0:C],
            )
        W = sb.tile([128, DCH, C], bf16)
        nc.scalar.mul(W[:], pt_ps[:], -2.0)

        # ||p||^2 in f32 via ACT square-accumulate (consistent with the
        # bf16 protos used in the Gram)
        pn_dump = sb.tile([C, D], bf16)
        pn_col = sb.tile([C, 1], f32)
        nc.scalar.activation(
            pn_dump[:], psb[:], AF.Square, accum_out=pn_col[:]
        )

        # ---------------- ||q||^2 squares (per chunk, DVE) ---------------
        qsq = sb.tile([128, DCH, QL], bf16)
        for k in range(DCH):
            nc.vector.tensor_mul(qsq[:, k], q16[:, k], q16[:, k])

        # ---------------- Gram + ||q||^2, fused in one PSUM group --------
        # s_ps[c, q] = sum_k ( W_k^T q_k  +  ones^T qsq_k )
        #            = -2 q.p + ||q||^2
        s_ps = ps.tile([C, QL], f32)
        for k in range(DCH):
            nc.tensor.matmul(
                s_ps[:], W[:, k], q16[:, k], start=(k == 0), stop=False
            )
            nc.tensor.matmul(
                s_ps[:], ones64[:], qsq[:, k], start=False,
                stop=(k == DCH - 1),
            )

        # ---------------- sqrt(+||p||^2), negate, store ------------------
        lt = sb.tile([C, QL], f32)
        nc.scalar.activation(lt[:], s_ps[:], AF.Sqrt, bias=pn_col[:, 0:1])
        nc.vector.tensor_scalar_mul(lt[:], lt[:], -1.0)
        nc.scalar.dma_start(out[:, :], lt[:])


def _build():
    if "nc" in _CACHE:
        return _CACHE["nc"]
    from concourse import bacc, mybir, tile

    f32 = mybir.dt.float32
    bf16 = mybir.dt.bfloat16
    fp8 = mybir.dt.float8e4
    nc = bacc.Bacc(
        "TRN2",
        target_bir_lowering=False,
        debug=False,
        enable_asserts=False,
        num_devices=NCORES,
    )
    sup = nc.dram_tensor("sup", [128, SCH * D], fp8, kind="ExternalInput").ap()
    qt = nc.dram_tensor("qt", [128, DCH * QL], bf16, kind="ExternalInput").ap()
    oh_in = nc.dram_tensor("oh", [128, SCH * C], fp8, kind="ExternalInput").ap()
    out = nc.dram_tensor("out", [C, QL], f32, kind="ExternalOutput").ap()
    with tile.TileContext(nc) as tc:
        _emit(nc, tc, sup, qt, oh_in, out)
    nc.compile()
    _CACHE["nc"] = nc
    return nc


def _onehot():
    import ml_dtypes

    # oh[p, j, c] = 1 iff class c owns support row j*128 + p,
    # i.e. c == 2*j + p//64
    p = np.arange(128)[:, None, None]
    j = np.arange(SCH)[None, :, None]
    c = np.arange(C)[None, None, :]
    oh = (c == 2 * j + p // 64).astype(ml_dtypes.float8_e4m3)
    return np.ascontiguousarray(oh.reshape(128, SCH * C))


def _shard(embeddings):
    import ml_dtypes

    emb = np.asarray(embeddings, dtype=np.float32).reshape(C, 2 * S, D)
    # support: [C*S, D] -> swizzled [128, SCH, D] (row p of chunk j =
    # support row j*128+p), fp8 on the wire
    sup = emb[:, :S, :].reshape(SCH, 128, D).transpose(1, 0, 2)
    sup = np.ascontiguousarray(
        sup.astype(ml_dtypes.float8_e4m3).reshape(128, SCH * D)
    )
    oh = _onehot()
    in_maps = []
    for i in range(NCORES):
        q = emb[CL * i : CL * (i + 1), S:, :].reshape(QL, D)
        # Q^T [D, QL] -> swizzled [128, DCH, QL] bf16
        qt_i = q.T.reshape(DCH, 128, QL).transpose(1, 0, 2)
        qt_i = np.ascontiguousarray(
            qt_i.astype(ml_dtypes.bfloat16).reshape(128, DCH * QL)
        )
        in_maps.append({"sup": sup, "qt": qt_i, "oh": oh})
    return in_maps


def kernel(embeddings_stacked, n_classes, n_support, **_unused):
    assert int(n_classes) == C and int(n_support) == S
    emb = np.asarray(embeddings_stacked)
    assert emb.shape == (C * 2 * S, D), emb.shape

    from concourse import bass_utils

    nc = _build()
    in_maps = _shard(emb)
    res = bass_utils.run_bass_kernel_spmd(nc, in_maps, core_ids=list(range(NCORES)))
    logits = np.empty((C * S, C), dtype=np.float32)
    for i in range(NCORES):
        logits[QL * i : QL * (i + 1), :] = res.results[i]["out"].T
    return logits


if __name__ == "__main__":
    rng = np.random.default_rng(0)
    emb = rng.standard_normal((C * 2 * S, D), dtype=np.float32)
    got = kernel(emb, C, S)
    print("kernel output", got.shape, got.dtype)


# revision 31
# speedup vs baseline: 1.0992x; 1.0992x over previous
"""Trainium2 Bass kernel for BaselineProtonet (retrieval_knn).

logits[q, c] = -||query_q - proto_c||_2
  proto_c = mean of 64 support embeddings of class c
  embeddings_stacked: [64 classes * (64 support + 64 query), 1024] f32

Sharding (8 cores): query-sharded, support-replicated. Core i owns query
rows 512i..512(i+1); every core receives the full support set (fp8 on
the wire) and computes all 64 prototypes locally on the TensorEngine, so
no cross-core collective is needed (a ncfw collective costs ~50us of
control latency in this runtime, far more than the extra DMA).

Host-side shard prep (layout/encoding only, no arithmetic): support is
pre-swizzled to the exact SBUF layout (contiguous per-partition runs so
HWDGE descriptor generation is cheap) and encoded fp8e4m3; queries are
transposed to feature-major (d on partitions) and encoded bf16.

Per core:
  protos   : 64 one-hot matmuls (fp8) accumulate class sums -> PSUM
             [64,1024] f32, scaled 1/64 on evacuation -> bf16 prototypes
  P^T      : 8 PE transposes -> W = -2*P^T (bf16)
  ||p||^2  : DVE square + reduce on prototypes -> [64,1] f32, added
             per-partition (class) via the ACT sqrt bias
  ||q||^2  : DVE squares + ones-stationary colsum matmuls -> [1,512]
             f32, added via a K=1 fp32 matmul broadcast over classes
  Gram     : 8 accumulating matmuls lhsT=W chunk, rhs=Q^T chunk (bf16)
  logits   : -sqrt(dist^2) via ACT sqrt(+bias) and DVE negate,
             output [64, 512] (class-major); host transposes/concats.
PE is pre-warmed with dummy matmuls during the DMA wait (HAM clock gate)
and the sqrt ACT table is preloaded by a dummy activation.
"""

import numpy as np

C = 64          # classes
S = 64          # support per class (== queries per class)
D = 1024        # embedding dim
NCORES = 8
CL = C // NCORES            # 8 classes per core's query shard
QL = CL * S                 # 512 query rows per core
DCH = D // 128              # 8 d-chunks
SCH = (C * S) // 128        # 32 support row chunks (full support)

_CACHE = {}


def _emit(nc, tc, sup, qt, oh_in, out):
    """Emit the per-core tile program.

    sup:   [128, SCH*D] fp8 DRAM  (full support, swizzled: row p holds
                                   sup[j*128+p, :] for j = 0..31)
    qt:    [128, DCH*QL] bf16 DRAM (queries, swizzled feature-major)
    oh_in: [128, SCH*C] fp8 DRAM  (one-hot class masks per row chunk)
    out:   [C, QL] f32 DRAM       (negated distances, class-major)
    """
    from concourse import masks, mybir

    f32 = mybir.dt.float32
    bf16 = mybir.dt.bfloat16
    fp8 = mybir.dt.float8e4
    AF = mybir.ActivationFunctionType

    with (
        tc.tile_pool(name="sb", bufs=1) as sb,
        tc.tile_pool(name="ps", bufs=1, space="PSUM") as ps,
    ):
        # warm the PE clock first-thing (HAM gate needs ~3.5us of busy
        # before the real matmuls; deps are a single DVE memset)
        wm_in = sb.tile([128, 512], bf16)
        nc.vector.memset(wm_in[:], 0.0)
        wm_ps = ps.tile([128, 512], f32)
        for _ in range(8):
            nc.tensor.matmul(
                wm_ps[:], wm_in[:, 0:128], wm_in[:], start=True, stop=True
            )

        # ---------------- input DMAs (one sync-ring FIFO: one-hot, the
        # support stream, then query quarters -- slices drain in order;
        # the prototype matmuls track the support stream and the
        # Gram/norm matmuls track the query stream)
        oh = sb.tile([128, SCH, C], fp8)
        nc.sync.dma_start(oh[:], oh_in[:, :].rearrange("p (c k) -> p c k", c=SCH))
        s8 = sb.tile([128, SCH, D], fp8)
        for b in range(8):
            nc.sync.dma_start(
                s8[:, 4 * b : 4 * (b + 1)],
                sup[:, 4 * b * D : 4 * (b + 1) * D].rearrange(
                    "p (c d) -> p c d", c=4
                ),
            )
        q16 = sb.tile([128, DCH, QL], bf16)
        for h in range(4):
            nc.sync.dma_start(
                q16[:, 2 * h : 2 * (h + 1)],
                qt[:, 2 * h * QL : 2 * (h + 1) * QL].rearrange(
                    "p (k q) -> p k q", k=2
                ),
            )

        # ---------------- constants -------------------------------------
        ident = sb.tile([128, 128], bf16)
        masks.make_identity(nc, ident[:])
        ones64 = sb.tile([128, C], bf16)
        nc.gpsimd.memset(ones64[:], 1.0)

        # preload the sqrt ACT table set off the critical path
        warm_sq = sb.tile([1, 1], f32)
        nc.gpsimd.memset(warm_sq[:], 1.0)
        nc.scalar.activation(warm_sq[:], warm_sq[:], AF.Sqrt)

        # ---------------- prototypes (all 64 classes) -------------------
        # col-paired fp8 matmuls: the d-halves of each row chunk run
        # concurrently on disjoint column halves of the PE array
        # (rows 0..63 of p_ps2 = class sums for d 0..511, rows 64..127 =
        # class sums for d 512..1023)
        p_ps2 = ps.tile([128, 512], f32)
        for j in range(SCH):
            for h in range(2):
                nc.tensor.matmul(
                    p_ps2[64 * h : 64 * (h + 1), :],
                    oh[:, j],
                    s8[:, j, 512 * h : 512 * (h + 1)],
                    start=(j == 0),
                    stop=(j == SCH - 1),
                    tile_position=(0, 64 * h),
                    # the sim's psum group tracker is not partition-aware;
                    # h=0 carries the group accounting for the bank
                    skip_group_check=(h == 1),
                )
        # evacuate the two halves in parallel on DVE and ACT
        psb = sb.tile([C, D], bf16)
        nc.vector.tensor_scalar_mul(psb[:, 0:512], p_ps2[0:64, :], 1.0 / S)
        nc.scalar.mul(psb[:, 512:1024], p_ps2[64:128, :], 1.0 / S)

        # ---------------- W = -2 * P^T (bf16, per-chunk ACT evac) --------
        pt_ps = ps.tile([128, DCH * C], bf16)  # chunk k at cols 64k..64k+64
        W = sb.tile([128, DCH, C], bf16)
        for k in range(DCH):
            nc.tensor.transpose(
                pt_ps[:, C * k : C * (k + 1)],
                psb[:, 128 * k : 128 * (k + 1)],
                ident[0:C, 0:C],
            )
        for k in range(DCH):
            nc.scalar.mul(W[:, k], pt_ps[:, C * k : C * (k + 1)], -2.0)

        # ---------------- ||q||^2 squares (per chunk, DVE) ---------------
        qsq = sb.tile([128, DCH, QL], bf16)
        for k in range(DCH):
            nc.vector.tensor_mul(qsq[:, k], q16[:, k], q16[:, k])

        # ------- Gram + ||q||^2, col-paired into two PSUM halves ---------
        # rows 0..63 accumulate even d-chunks, rows 64..127 odd d-chunks;
        # both column halves of the PE array run concurrently
        s_ps2 = ps.tile([128, QL], f32)
        for kp in range(DCH // 2):
            for par, k in ((0, 2 * kp), (1, 2 * kp + 1)):
                nc.tensor.matmul(
                    s_ps2[64 * par : 64 * (par + 1), :],
                    W[:, k],
                    q16[:, k],
                    start=(kp == 0),
                    stop=False,
                    tile_position=(0, 64 * par),
                    skip_group_check=(par == 1),
                )
            for par, k in ((0, 2 * kp), (1, 2 * kp + 1)):
                nc.tensor.matmul(
                    s_ps2[64 * par : 64 * (par + 1), :],
                    ones64[:],
                    qsq[:, k],
                    start=False,
                    stop=(kp == DCH // 2 - 1),
                    tile_position=(0, 64 * par),
                    skip_group_check=(par == 1),
                )

        # ||p||^2 in f32 via ACT square-accumulate (consistent with the
        # bf16 protos used in the Gram)
        pn_dump = sb.tile([C, D], bf16)
        pn_col = sb.tile([C, 1], f32)
        nc.scalar.activation(
            pn_dump[:], psb[:], AF.Square, accum_out=pn_col[:]
        )

        # ---------------- add halves, sqrt(+||p||^2), negate, store ------
        sh = sb.tile([C, QL], f32)
        nc.scalar.copy(sh[:], s_ps2[64:128, :])
        sadd = sb.tile([C, QL], f32)
        nc.vector.tensor_add(sadd[:], s_ps2[0:64, :], sh[:])
        lt = sb.tile([C, QL], f32)
        nc.scalar.activation(lt[:], sadd[:], AF.Sqrt, bias=pn_col[:, 0:1])
        nc.vector.tensor_scalar_mul(lt[:], lt[:], -1.0)
        nc.scalar.dma_start(out[:, :], lt[:])


def _build():
    if "nc" in _CACHE:
        return _CACHE["nc"]
    from concourse import bacc, mybir, tile

    f32 = mybir.dt.float32
    bf16 = mybir.dt.bfloat16
    fp8 = mybir.dt.float8e4
    nc = bacc.Bacc(
        "TRN2",
        target_bir_lowering=False,
        debug=False,
        enable_asserts=False,
        num_devices=NCORES,
    )
    sup = nc.dram_tensor("sup", [128, SCH * D], fp8, kind="ExternalInput").ap()
    qt = nc.dram_tensor("qt", [128, DCH * QL], bf16, kind="ExternalInput").ap()
    oh_in = nc.dram_tensor("oh", [128, SCH * C], fp8, kind="ExternalInput").ap()
    out = nc.dram_tensor("out", [C, QL], f32, kind="ExternalOutput").ap()
    with tile.TileContext(nc) as tc:
        _emit(nc, tc, sup, qt, oh_in, out)
    nc.compile()
    _CACHE["nc"] = nc
    return nc


def _onehot():
    import ml_dtypes

    # oh[p, j, c] = 1 iff class c owns support row j*128 + p,
    # i.e. c == 2*j + p//64
    p = np.arange(128)[:, None, None]
    j = np.arange(SCH)[None, :, None]
    c = np.arange(C)[None, None, :]
    oh = (c == 2 * j + p // 64).astype(ml_dtypes.float8_e4m3)
    return np.ascontiguousarray(oh.reshape(128, SCH * C))


def _shard(embeddings):
    import ml_dtypes

    emb = np.asarray(embeddings, dtype=np.float32).reshape(C, 2 * S, D)
    # support: [C*S, D] -> swizzled [128, SCH, D] (row p of chunk j =
    # support row j*128+p), fp8 on the wire
    sup = emb[:, :S, :].reshape(SCH, 128, D).transpose(1, 0, 2)
    sup = np.ascontiguousarray(
        sup.astype(ml_dtypes.float8_e4m3).reshape(128, SCH * D)
    )
    oh = _onehot()
    in_maps = []
    for i in range(NCORES):
        q = emb[CL * i : CL * (i + 1), S:, :].reshape(QL, D)
        # Q^T [D, QL] -> swizzled [128, DCH, QL] bf16
        qt_i = q.T.reshape(DCH, 128, QL).transpose(1, 0, 2)
        qt_i = np.ascontiguousarray(
            qt_i.astype(ml_dtypes.bfloat16).reshape(128, DCH * QL)
        )
        in_maps.append({"sup": sup, "qt": qt_i, "oh": oh})
    return in_maps


def kernel(embeddings_stacked, n_classes, n_support, **_unused):
    assert int(n_classes) == C and int(n_support) == S
    emb = np.asarray(embeddings_stacked)
    assert emb.shape == (C * 2 * S, D), emb.shape

    from concourse import bass_utils

    nc = _build()
    in_maps = _shard(emb)
    res = bass_utils.run_bass_kernel_spmd(nc, in_maps, core_ids=list(range(NCORES)))
    logits = np.empty((C * S, C), dtype=np.float32)
    for i in range(NCORES):
        logits[QL * i : QL * (i + 1), :] = res.results[i]["out"].T
    return logits


if __name__ == "__main__":
    rng = np.random.default_rng(0)
    emb = rng.standard_normal((C * 2 * S, D), dtype=np.float32)
    got = kernel(emb, C, S)
    print("kernel output", got.shape, got.dtype)


# revision 33
# speedup vs baseline: 1.1392x; 1.0363x over previous
"""Trainium2 Bass kernel for BaselineProtonet (retrieval_knn).

logits[q, c] = -||query_q - proto_c||_2
  proto_c = mean of 64 support embeddings of class c
  embeddings_stacked: [64 classes * (64 support + 64 query), 1024] f32

Sharding (8 cores): query-sharded, support-replicated. Core i owns query
rows 512i..512(i+1); every core receives the full support set (fp8 on
the wire) and computes all 64 prototypes locally on the TensorEngine, so
no cross-core collective is needed (a ncfw collective costs ~50us of
control latency in this runtime, far more than the extra DMA).

Host-side shard prep (layout/encoding only, no arithmetic): support is
pre-swizzled to the exact SBUF layout (contiguous per-partition runs so
HWDGE descriptor generation is cheap) and encoded fp8e4m3; queries are
transposed to feature-major (d on partitions) and encoded bf16.

Per core:
  protos   : 64 one-hot matmuls (fp8) accumulate class sums -> PSUM
             [64,1024] f32, scaled 1/64 on evacuation -> bf16 prototypes
  P^T      : 8 PE transposes -> W = -2*P^T (bf16)
  ||p||^2  : DVE square + reduce on prototypes -> [64,1] f32, added
             per-partition (class) via the ACT sqrt bias
  ||q||^2  : DVE squares + ones-stationary colsum matmuls -> [1,512]
             f32, added via a K=1 fp32 matmul broadcast over classes
  Gram     : 8 accumulating matmuls lhsT=W chunk, rhs=Q^T chunk (bf16)
  logits   : -sqrt(dist^2) via ACT sqrt(+bias) and DVE negate,
             output [64, 512] (class-major); host transposes/concats.
PE is pre-warmed with dummy matmuls during the DMA wait (HAM clock gate)
and the sqrt ACT table is preloaded by a dummy activation.
"""

import numpy as np

C = 64          # classes
S = 64          # support per class (== queries per class)
D = 1024        # embedding dim
NCORES = 8
CL = C // NCORES            # 8 classes per core's query shard
QL = CL * S                 # 512 query rows per core
DCH = D // 128              # 8 d-chunks
SCH = (C * S) // 128        # 32 support row chunks (full support)

_CACHE = {}


def _emit(nc, tc, sup, qt, oh_in, out):
    """Emit the per-core tile program.

    sup:   [128, SCH*D] fp8 DRAM  (full support, swizzled: row p holds
                                   sup[j*128+p, :] for j = 0..31)
    qt:    [128, DCH*QL] bf16 DRAM (queries, swizzled feature-major)
    oh_in: [128, SCH*C] fp8 DRAM  (one-hot class masks per row chunk)
    out:   [C, QL] f32 DRAM       (negated distances, class-major)
    """
    from concourse import masks, mybir

    f32 = mybir.dt.float32
    bf16 = mybir.dt.bfloat16
    fp8 = mybir.dt.float8e4
    AF = mybir.ActivationFunctionType

    with (
        tc.tile_pool(name="sb", bufs=1) as sb,
        tc.tile_pool(name="ps", bufs=1, space="PSUM") as ps,
    ):
        # warm the PE clock first-thing (HAM gate needs ~3.5us of busy
        # before the real matmuls; deps are a single DVE memset)
        wm_in = sb.tile([128, 512], bf16)
        nc.vector.memset(wm_in[:], 0.0)
        wm_ps = ps.tile([128, 512], f32)
        for _ in range(8):
            nc.tensor.matmul(
                wm_ps[:], wm_in[:, 0:128], wm_in[:], start=True, stop=True
            )

        # ---------------- input DMAs (one sync-ring FIFO: one-hot, the
        # support stream, then query quarters -- slices drain in order;
        # the prototype matmuls track the support stream and the
        # Gram/norm matmuls track the query stream)
        oh = sb.tile([128, SCH, C], fp8)
        nc.sync.dma_start(oh[:], oh_in[:, :].rearrange("p (c k) -> p c k", c=SCH))
        s8 = sb.tile([128, SCH, D], fp8)
        for b in range(8):
            nc.sync.dma_start(
                s8[:, 4 * b : 4 * (b + 1)],
                sup[:, 4 * b * D : 4 * (b + 1) * D].rearrange(
                    "p (c d) -> p c d", c=4
                ),
            )
        q16 = sb.tile([128, DCH, QL], bf16)
        for h in range(4):
            nc.sync.dma_start(
                q16[:, 2 * h : 2 * (h + 1)],
                qt[:, 2 * h * QL : 2 * (h + 1) * QL].rearrange(
                    "p (k q) -> p k q", k=2
                ),
            )

        # ---------------- constants -------------------------------------
        ident = sb.tile([128, 128], bf16)
        masks.make_identity(nc, ident[:])
        ones64 = sb.tile([128, C], bf16)
        nc.gpsimd.memset(ones64[:], 1.0)

        # preload the sqrt ACT table set off the critical path
        warm_sq = sb.tile([1, 1], f32)
        nc.gpsimd.memset(warm_sq[:], 1.0)
        nc.scalar.activation(warm_sq[:], warm_sq[:], AF.Sqrt)

        # ---------------- prototypes (all 64 classes) -------------------
        # col-paired fp8 matmuls: the d-halves of each row chunk run
        # concurrently on disjoint column halves of the PE array
        # (rows 0..63 of p_ps2 = class sums for d 0..511, rows 64..127 =
        # class sums for d 512..1023)
        p_ps2 = ps.tile([128, 512], f32)
        for j in range(SCH):
            for h in range(2):
                nc.tensor.matmul(
                    p_ps2[64 * h : 64 * (h + 1), :],
                    oh[:, j],
                    s8[:, j, 512 * h : 512 * (h + 1)],
                    start=(j == 0),
                    stop=(j == SCH - 1),
                    tile_position=(0, 64 * h),
                    # the sim's psum group tracker is not partition-aware;
                    # h=0 carries the group accounting for the bank
                    skip_group_check=(h == 1),
                )
        # evacuate the two halves in parallel on DVE and ACT
        psb = sb.tile([C, D], bf16)
        nc.vector.tensor_scalar_mul(psb[:, 0:512], p_ps2[0:64, :], 1.0 / S)
        nc.scalar.mul(psb[:, 512:1024], p_ps2[64:128, :], 1.0 / S)

        # ---------------- W = -2 * P^T (bf16, ACT evac) ------------------
        pt_ps = ps.tile([128, DCH * C], bf16)  # chunk k at cols 64k..64k+64
        W = sb.tile([128, DCH, C], bf16)
        for k in range(DCH):
            nc.tensor.transpose(
                pt_ps[:, C * k : C * (k + 1)],
                psb[:, 128 * k : 128 * (k + 1)],
                ident[0:C, 0:C],
            )
        nc.scalar.mul(W[:], pt_ps[:], -2.0)

        # ---------------- ||q||^2 squares (per chunk, DVE) ---------------
        qsq = sb.tile([128, DCH, QL], bf16)
        for k in range(DCH):
            nc.vector.tensor_mul(qsq[:, k], q16[:, k], q16[:, k])

        # ------- Gram + ||q||^2, col-paired into two PSUM halves ---------
        # rows 0..63 accumulate even d-chunks, rows 64..127 odd d-chunks;
        # both column halves of the PE array run concurrently
        s_ps2 = ps.tile([128, QL], f32)
        for kp in range(DCH // 2):
            for par, k in ((0, 2 * kp), (1, 2 * kp + 1)):
                nc.tensor.matmul(
                    s_ps2[64 * par : 64 * (par + 1), :],
                    W[:, k],
                    q16[:, k],
                    start=(kp == 0),
                    stop=False,
                    tile_position=(0, 64 * par),
                    skip_group_check=(par == 1),
                )
            for par, k in ((0, 2 * kp), (1, 2 * kp + 1)):
                nc.tensor.matmul(
                    s_ps2[64 * par : 64 * (par + 1), :],
                    ones64[:],
                    qsq[:, k],
                    start=False,
                    stop=(kp == DCH // 2 - 1),
                    tile_position=(0, 64 * par),
                    skip_group_check=(par == 1),
                )

        # ||p||^2 in f32 on DVE (square + reduce; consistent with the
        # bf16 protos used in the Gram). Off the ACT critical path.
        pn_dump = sb.tile([C, D], bf16)
        pn_col = sb.tile([C, 1], f32)
        nc.vector.tensor_mul(pn_dump[:], psb[:], psb[:])
        nc.vector.tensor_reduce(
            pn_col[:], pn_dump[:], axis=mybir.AxisListType.X, op=mybir.AluOpType.add
        )

        # ------- add halves, sqrt(+||p||^2), negate, store (2 q-halves
        # pipelined across ACT/DVE/DMA) -----------------------------------
        sh = sb.tile([C, QL], f32)
        sadd = sb.tile([C, QL], f32)
        lt = sb.tile([C, QL], f32)
        for hq in range(2):
            s = slice(256 * hq, 256 * (hq + 1))
            nc.scalar.copy(sh[:, s], s_ps2[64:128, s])
            nc.vector.tensor_add(sadd[:, s], s_ps2[0:64, s], sh[:, s])
            nc.scalar.activation(lt[:, s], sadd[:, s], AF.Sqrt, bias=pn_col[:, 0:1])
            nc.vector.tensor_scalar_mul(lt[:, s], lt[:, s], -1.0)
            nc.scalar.dma_start(out[:, s], lt[:, s])


def _build():
    if "nc" in _CACHE:
        return _CACHE["nc"]
    from concourse import bacc, mybir, tile

    f32 = mybir.dt.float32
    bf16 = mybir.dt.bfloat16
    fp8 = mybir.dt.float8e4
    nc = bacc.Bacc(
        "TRN2",
        target_bir_lowering=False,
        debug=False,
        enable_asserts=False,
        num_devices=NCORES,
    )
    sup = nc.dram_tensor("sup", [128, SCH * D], fp8, kind="ExternalInput").ap()
    qt = nc.dram_tensor("qt", [128, DCH * QL], bf16, kind="ExternalInput").ap()
    oh_in = nc.dram_tensor("oh", [128, SCH * C], fp8, kind="ExternalInput").ap()
    out = nc.dram_tensor("out", [C, QL], f32, kind="ExternalOutput").ap()
    with tile.TileContext(nc) as tc:
        _emit(nc, tc, sup, qt, oh_in, out)
    nc.compile()
    _CACHE["nc"] = nc
    return nc


def _onehot():
    import ml_dtypes

    # oh[p, j, c] = 1 iff class c owns support row j*128 + p,
    # i.e. c == 2*j + p//64
    p = np.arange(128)[:, None, None]
    j = np.arange(SCH)[None, :, None]
    c = np.arange(C)[None, None, :]
    oh = (c == 2 * j + p // 64).astype(ml_dtypes.float8_e4m3)
    return np.ascontiguousarray(oh.reshape(128, SCH * C))


def _shard(embeddings):
    import ml_dtypes

    emb = np.asarray(embeddings, dtype=np.float32).reshape(C, 2 * S, D)
    # support: [C*S, D] -> swizzled [128, SCH, D] (row p of chunk j =
    # support row j*128+p), fp8 on the wire
    sup = emb[:, :S, :].reshape(SCH, 128, D).transpose(1, 0, 2)
    sup = np.ascontiguousarray(
        sup.astype(ml_dtypes.float8_e4m3).reshape(128, SCH * D)
    )
    oh = _onehot()
    in_maps = []
    for i in range(NCORES):
        q = emb[CL * i : CL * (i + 1), S:, :].reshape(QL, D)
        # Q^T [D, QL] -> swizzled [128, DCH, QL] bf16
        qt_i = q.T.reshape(DCH, 128, QL).transpose(1, 0, 2)
        qt_i = np.ascontiguousarray(
            qt_i.astype(ml_dtypes.bfloat16).reshape(128, DCH * QL)
        )
        in_maps.append({"sup": sup, "qt": qt_i, "oh": oh})
    return in_maps


def kernel(embeddings_stacked, n_classes, n_support, **_unused):
    assert int(n_classes) == C and int(n_support) == S
    emb = np.asarray(embeddings_stacked)
    assert emb.shape == (C * 2 * S, D), emb.shape

    from concourse import bass_utils

    nc = _build()
    in_maps = _shard(emb)
    res = bass_utils.run_bass_kernel_spmd(nc, in_maps, core_ids=list(range(NCORES)))
    logits = np.empty((C * S, C), dtype=np.float32)
    for i in range(NCORES):
        logits[QL * i : QL * (i + 1), :] = res.results[i]["out"].T
    return logits


if __name__ == "__main__":
    rng = np.random.default_rng(0)
    emb = rng.standard_normal((C * 2 * S, D), dtype=np.float32)
    got = kernel(emb, C, S)
    print("kernel output", got.shape, got.dtype)
